# revision 28
# baseline (speedup 1.0000x reference)
"""Conv2d 3x3 (stride 1, pad 1) as implicit GEMM on 8 Trainium2 NeuronCores.

x: [32, 128, 56, 56] f32, W: [256, 128, 3, 3] f32 -> out: [32, 256, 56, 56] f32

Sharding: data-parallel over batch, 4 images per core (sharding_hint).

fp8 DoubleRow formulation (vs the previous bf16 kernel's 9 cycles/col, this
runs 5.5 PE cycles per output column per cout-half):
  - e4m3 residual split on the host: xh = Q(x), xl = Q((x-xh)*2^4),
    wh = Q(W*2^7), wl = Q((W*2^7-wh)*2^4), plus exactly power-of-2-rescaled
    copies whB = Q(wh/16), wlB = Q(wl/16) so every product lands on the common
    PSUM scale x*W*2^7 (host divides the output by 2^7 at the end — exact).
  - each DoubleRow matmul contracts TWO K=128 pairs at 0.5 cycles per output
    column: out += A_w^T A_x + B_w^T B_x. Per output tile (4 output rows,
    one cout half) 11 matmuls:
      m0..m8  (tap k): moving (xh_k | xl_k), stationary (wh_k | whB_k)
               -> full-precision x against fp8 W
      m9, m10: moving (xh_t, xh_t') for tap pairs (1,3), (5,7), stationary
               (wlB_t | wlB_t') -> W-residual correction on 4 of 9 taps; the
               other 5 taps' W-residuals are dropped. The tap-shifted xh
               copies are materialized host-side (a matmul moving AP whose
               pair slots overlap in SBUF fails walrus codegen)
    measured rel L2 err 0.0169 vs the f32 reference (tolerance 2e-2).
  - moving windows are flat 230-element runs over the padded 58-wide rows
    (j = r*58 + w); the 2 pad columns per row compute garbage that the
    PSUM->SBUF copy APs skip. In-DMA stays one padded fp8 image pair.
  - PSUM->SBUF copies convert to bf16 (halves out-DMA bytes; host upcasts)
    and alternate between the DVE and Activation engines; stores batch 14
    row groups into one [128, 3136] staging buffer -> one DMA per
    (image, cout half) to keep the serialized HWDGE/DMA devices off the
    critical path.
  - warmup matmul chain holds the PE p-state ramp as in the bf16 kernel.
"""

import sys

for _p in ("/opt/trn_rl_repo",):
    if _p not in sys.path:
        sys.path.insert(0, _p)

import numpy as np
import ml_dtypes

import concourse.bass as bass
import concourse.bacc as bacc
import concourse.mybir as mybir
from concourse import tile
from concourse.bass_utils import run_bass_kernel_spmd

N_CORES = 8
B = 32
B_PER_CORE = B // N_CORES  # 4
CIN = 128
COUT = 256
H = W_DIM = 56
HP = WP = 58  # padded
IMG = HP * WP  # 3364
ROWS = 4                # output rows per chain
NG = H // ROWS          # 14 row groups
NWIN = (ROWS - 1) * WP + W_DIM  # 230: flat window length per chain
COUT_TILES = COUT // 128  # 2
KEEP = (1, 3, 5, 7)     # taps with W-residual correction; pairs (1,3),(5,7)
NMM = 11                # matmuls per output tile
WCOLS = COUT_TILES * NMM * 2 * 128  # 5632 weight columns

X_SCALE = 128.0         # W pre-scale 2^7; host divides output by this
RES_SCALE = 16.0        # residual pre-scale 2^4

_NC_CACHE = None


def _tap_off(k: int) -> int:
    return (k // 3) * WP + (k % 3)


def build_nc(reps: int = 1, warm: int = 60) -> bass.Bass:
    # Bacc (not raw Bass): its compile() legalizes multi-wait instructions for
    # the 1-sync-wait-per-instruction encoding limit of this toolchain.
    nc = bacc.Bacc()
    xp = nc.dram_tensor(
        "xp", [B_PER_CORE, CIN, 2, IMG], mybir.dt.float8e4, kind="ExternalInput"
    )
    xq = nc.dram_tensor(
        "xq", [B_PER_CORE, CIN, 4, IMG], mybir.dt.float8e4, kind="ExternalInput"
    )
    wt = nc.dram_tensor("wt", [CIN, WCOLS], mybir.dt.float8e4, kind="ExternalInput")
    out = nc.dram_tensor(
        "out", [B_PER_CORE, COUT, H * W_DIM], mybir.dt.bfloat16, kind="ExternalOutput"
    )


    with tile.TileContext(nc) as tc:
        with (
            tc.tile_pool(name="wpool", bufs=1) as wpool,
            tc.tile_pool(name="xpool", bufs=1) as xpool,
            tc.tile_pool(name="stpool", bufs=4) as stpool,
            tc.tile_pool(name="pspool", bufs=7, space="PSUM") as pspool,
            tc.tile_pool(name="warmpool", bufs=1, space="PSUM") as warmpool,
        ):
            # Keep the PE p-state ramp warm while the first loads land: a
            # chain of dependency-free matmuls on a memset scratch tile
            # (memset on the otherwise-idle gpsimd engine so the chain can
            # start as early as possible).
            scratch = stpool.tile([128, 64], mybir.dt.bfloat16, name="warm_src", tag="wsrc")
            nc.gpsimd.memset(scratch, 0.0)
            warm_ps = warmpool.tile([64, 64], mybir.dt.float32, name="warm_ps", tag="wps")
            for _ in range(warm):
                nc.tensor.matmul(warm_ps, scratch[:, :64], scratch, start=True, stop=True)

            # All loads ride one explicitly-ordered ring (the shared HWDGE /
            # DMA devices drain FIFO, so issue order is arrival order):
            # weights for the first chains, then image-0 row chunks sized to
            # stay ahead of the group ladder, then whole images 1-3.
            # One weight TILE per chunk — a single big tile makes every
            # matmul wait on the LAST weight DMA through coarse dep tracking.
            WSPLITS = (0, 9 * 2 * 128, NMM * 2 * 128, (NMM + 9) * 2 * 128, WCOLS)
            w_tiles = []
            for lo, hi in zip(WSPLITS[:-1], WSPLITS[1:]):
                w_tiles.append(
                    (lo, hi, wpool.tile(
                        [CIN, hi - lo], mybir.dt.float8e4, name=f"w_sb{lo}", tag=f"w{lo}"
                    ))
                )

            def w_slice(col):
                for lo, hi, wtile in w_tiles:
                    if lo <= col and col + 256 <= hi:
                        return wtile[:, col - lo : col - lo + 256]
                raise AssertionError(col)

            x_sb, xq_sb = [], []
            for b in range(B_PER_CORE):
                x_sb.append(xpool.tile(
                    [CIN, 2, IMG], mybir.dt.float8e4, name=f"x_sb{b}", tag=f"x{b}"
                ))
                xq_sb.append(xpool.tile(
                    [CIN, 4, IMG], mybir.dt.float8e4, name=f"xq_sb{b}", tag=f"xq{b}"
                ))

            def load_x(b, lo, hi):
                nc.sync.dma_start(
                    x_sb[b][:, :, lo * WP : hi * WP], xp[b, :, :, lo * WP : hi * WP]
                )
                nc.sync.dma_start(
                    xq_sb[b][:, :, lo * WP : hi * WP], xq[b, :, :, lo * WP : hi * WP]
                )

            def load_w(i):
                nc.sync.dma_start(w_tiles[i][2], wt[:, WSPLITS[i] : WSPLITS[i + 1]])

            load_w(0)
            nc.sync.dma_start(
                x_sb[0][:, :, : 10 * WP], xp[0, :, :, : 10 * WP]
            )
            load_w(1)
            nc.sync.dma_start(
                xq_sb[0][:, :, : 10 * WP], xq[0, :, :, : 10 * WP]
            )
            load_w(2)
            load_w(3)
            B0SPLITS = (10, 19, 28, 37, 46, 55, HP)
            for lo, hi in zip(B0SPLITS[:-1], B0SPLITS[1:]):
                load_x(0, lo, hi)
            for b in range(1, B_PER_CORE):
                load_x(b, 0, HP)

            for _rep in range(reps):
              for b in range(B_PER_CORE):
                last_b = b == B_PER_CORE - 1
                # cout halves interleaved per group pair: phase-1 x rows are
                # consumed at half the rate, so image-0's streaming chunks
                # stay ahead of the PE.
                sts = [
                    stpool.tile([128, H * W_DIM], mybir.dt.bfloat16, name="st", tag="st")
                    for _ in range(COUT_TILES)
                ]
                for gp in range(NG // 2):
                    for c in range(COUT_TILES):
                        st = sts[c]
                        ps = pspool.tile([128, 512], mybir.dt.float32, name="ps", tag="ps")
                        for half in range(2):
                            g = 2 * gp + half
                            out_ps = ps[:, half * NWIN : (half + 1) * NWIN]
                            base = ROWS * g * WP
                            for m in range(NMM):
                                if m < 9:
                                    s = base + _tap_off(m)
                                    rhs = x_sb[b][:, :, s : s + NWIN]
                                else:
                                    sl = 2 * (m - 9)
                                    rhs = xq_sb[b][:, sl : sl + 2, base : base + NWIN]
                                lhsT = w_slice((c * NMM + m) * 2 * 128).rearrange(
                                    "p (two m) -> p two m", two=2
                                )
                                nc.tensor.matmul(
                                    out_ps,
                                    lhsT,
                                    rhs,
                                    start=(m == 0),
                                    stop=(m == NMM - 1),
                                    perf_mode=mybir.MatmulPerfMode.DoubleRow,
                                    skip_group_check=(half == 1),
                                )
                        # copy the chains (skipping the per-row pad columns)
                        # into the bf16 staging buffer; alternate engines.
                        # For the very last unit each half is copied and
                        # stored separately so the kernel tail is one small
                        # copy plus one short DMA.
                        final_unit = (
                            last_b and c == COUT_TILES - 1 and gp == NG // 2 - 1
                        )
                        gcol = gp * 2 * ROWS * W_DIM
                        if final_unit:
                            for half in range(2):
                                src = bass.AP(
                                    tensor=ps.tensor,
                                    offset=ps.offset + half * NWIN,
                                    ap=[list(ps.ap[0]), [WP, ROWS], [1, W_DIM]],
                                )
                                lo = gcol + half * ROWS * W_DIM
                                dst = bass.AP(
                                    tensor=st.tensor,
                                    offset=st.offset + lo,
                                    ap=[list(st.ap[0]), [W_DIM, ROWS], [1, W_DIM]],
                                )
                                if half == 0:
                                    nc.vector.tensor_copy(dst, src)
                                else:
                                    nc.scalar.activation(
                                        dst, src, mybir.ActivationFunctionType.Copy
                                    )
                                nc.sync.dma_start(
                                    out[b, c * 128 : (c + 1) * 128, lo : lo + ROWS * W_DIM],
                                    st[:, lo : lo + ROWS * W_DIM],
                                )
                        else:
                            src = bass.AP(
                                tensor=ps.tensor,
                                offset=ps.offset,
                                ap=[list(ps.ap[0]), [NWIN, 2], [WP, ROWS], [1, W_DIM]],
                            )
                            dst = bass.AP(
                                tensor=st.tensor,
                                offset=st.offset + gcol,
                                ap=[list(st.ap[0]), [ROWS * W_DIM, 2], [W_DIM, ROWS], [1, W_DIM]],
                            )
                            if c == 0:
                                nc.vector.tensor_copy(dst, src)
                            else:
                                nc.scalar.activation(
                                    dst, src, mybir.ActivationFunctionType.Copy
                                )
                            # last image: store in small pieces right behind
                            # the copies so the kernel tail stays short.
                            if last_b and gp in (1, 3, 5, 6):
                                lo = {1: 0, 3: 2, 5: 4, 6: 6}[gp] * 2 * ROWS * W_DIM
                                hi = (gp + 1) * 2 * ROWS * W_DIM
                                nc.sync.dma_start(
                                    out[b, c * 128 : (c + 1) * 128, lo:hi],
                                    st[:, lo:hi],
                                )
                if not last_b:
                    for c in range(COUT_TILES):
                        nc.sync.dma_start(
                            out[b, c * 128 : (c + 1) * 128, :], sts[c]
                        )
    nc.compile()
    return nc


def _get_nc() -> bass.Bass:
    global _NC_CACHE
    if _NC_CACHE is None:
        _NC_CACHE = build_nc()
    return _NC_CACHE


def _prep_inputs(x: np.ndarray, W: np.ndarray):
    e4 = ml_dtypes.float8_e4m3
    x = np.asarray(x, dtype=np.float32)
    W = np.asarray(W, dtype=np.float32)

    xpad = np.zeros((B, CIN, HP, WP), dtype=np.float32)
    xpad[:, :, 1 : 1 + H, 1 : 1 + W_DIM] = x
    xh8 = xpad.astype(e4)
    xl8 = ((xpad - xh8.astype(np.float32)) * RES_SCALE).astype(e4)
    # [B, CIN, 2, IMG]: slot 0 = xh, slot 1 = xl
    xhl = np.stack(
        [xh8.reshape(B, CIN, IMG), xl8.reshape(B, CIN, IMG)], axis=2
    )
    # [B, CIN, 4, IMG]: tap-shifted xh copies for the W-residual matmuls
    # (slot i = xh shifted left by _tap_off(KEEP[i]))
    xh_flat = xh8.reshape(B, CIN, IMG)
    xsh = np.zeros((B, CIN, 4, IMG), dtype=e4)
    for i, t in enumerate(KEEP):
        d = _tap_off(t)
        xsh[:, :, i, : IMG - d] = xh_flat[:, :, d:]

    Ws = W * X_SCALE
    wh8 = Ws.astype(e4)
    whf = wh8.astype(np.float32)
    wl8 = ((Ws - whf) * RES_SCALE).astype(e4)
    whB8 = (whf / RES_SCALE).astype(e4)
    wlB8 = (wl8.astype(np.float32) / RES_SCALE).astype(e4)

    # wt[ci, ((c*11 + m)*2 + slot)*128 + j] with cout = c*128 + j
    wtbuf = np.zeros((CIN, WCOLS), dtype=e4)
    for c in range(COUT_TILES):
        co = slice(c * 128, (c + 1) * 128)
        for m in range(NMM):
            col = (c * NMM + m) * 2 * 128
            if m < 9:
                kh, kw = divmod(m, 3)
                wtbuf[:, col : col + 128] = wh8[co, :, kh, kw].T
                wtbuf[:, col + 128 : col + 256] = whB8[co, :, kh, kw].T
            else:
                t0, t1 = KEEP[2 * (m - 9)], KEEP[2 * (m - 9) + 1]
                wtbuf[:, col : col + 128] = wlB8[co, :, t0 // 3, t0 % 3].T
                wtbuf[:, col + 128 : col + 256] = wlB8[co, :, t1 // 3, t1 % 3].T

    in_maps = []
    for cidx in range(N_CORES):
        in_maps.append(
            {
                "xp": np.ascontiguousarray(
                    xhl[cidx * B_PER_CORE : (cidx + 1) * B_PER_CORE]
                ),
                "xq": np.ascontiguousarray(
                    xsh[cidx * B_PER_CORE : (cidx + 1) * B_PER_CORE]
                ),
                "wt": wtbuf,
            }
        )
    return in_maps


def kernel_run(x: np.ndarray, W: np.ndarray, **spmd_kwargs):
    """Run the conv and return (output, BassKernelResults)."""
    in_maps = _prep_inputs(x, W)
    res = run_bass_kernel_spmd(
        _get_nc(), in_maps, core_ids=list(range(N_CORES)), **spmd_kwargs
    )
    out = np.concatenate(
        [
            np.asarray(res.results[cidx]["out"])
            .astype(np.float32)
            .reshape(B_PER_CORE, COUT, H, W_DIM)
            for cidx in range(N_CORES)
        ],
        axis=0,
    )
    out *= np.float32(1.0 / X_SCALE)
    return out, res


def kernel(x: np.ndarray, W: np.ndarray) -> np.ndarray:
    out, _ = kernel_run(x, W)
    return out
